# revision 68
# baseline (speedup 1.0000x reference)
"""GCN message-passing Bass kernel for TRN2 (8 cores), v3.

Math: delta = segment_sum(w_e * x[src_e]) @ W^T  (transform after aggregate).

Sharding: targets split across 8 cores (12500 each), then into 4 shards of
3125 targets per core (so gather indices fit int16).  Per shard, x rows the
shard actually reads are host-compacted and stored as 512-byte PAIRS; a
host-side greedy matching pairs up rows that some target reads together, so
one DMA descriptor often delivers two useful edge messages.  Per target the
edge list becomes a descriptor list (pair descs carry two weighted halves,
singleton descs carry one, the unused half gets weight 0).

Targets are sorted by descriptor count (descending) into 128-target blocks;
block j gets d_j descriptor columns in a [128 lanes, S] grid.  One Q7
`dma_gather` instruction fills up to 31 columns (31*128 int16 indices),
amortizing the 994 ns SWDGE fixed cost; dst[i%128, i//128] = src[idx[i]]
matches the grid when the index list is ordered column-major.

The weight multiply is split ~46/54 between Pool and DVE tensor_tensor
(cost model: Pool ~2.0 ns/elem, DVE ~1.1 ns/elem) so Pool-busy ≈ DVE-busy ≈
DMA-busy; DVE reduces each block to agg [128, 64] (consecutive equal-width
blocks fused into one reduce), PE transposes and applies W^T, the Activation
engine does PSUM->SBUF copies, and staged `dma_scatter_add` calls (int16
core-local target ids, trailing -1 padding) write rows into the pre-zeroed
output.  Groups run in reverse block order per shard and gathers are issued
3 groups ahead on rotating SWDGE queues so descriptor generation overlaps
transfers; the first gathers are halved to fill the pipeline faster.

Device limits found empirically: dma_gather/dma_scatter_add hang above 31
destination columns, and single_packet=True deadlocks when descriptors
exceed the SWDGE ring (1024), so all use single_packet=False.
"""

import math
from contextlib import ExitStack

import ml_dtypes
import numpy as np

import concourse.bass as bass
import concourse.bacc as bacc
import concourse.mybir as mybir
import concourse.tile as tile
from concourse.bass_utils import run_bass_kernel_spmd

P = 128
N_CORES = 8
NSH = 4                # target shards per core (int16 gather-index limit)
D = 64
D2 = 2 * D             # pair row: 128 floats = 512 B
GCAP = 31              # max descriptor columns per dma_gather (device limit)
F32 = mybir.dt.float32
I16 = mybir.dt.int16


def _wrap16(v):
    """Index list -> [16, n/16] wrap (element i at [i%16, i//16]), tiled to
    128 partitions (8 Q7 core replicas)."""
    v = np.asarray(v, dtype=np.int16)
    n = len(v)
    assert n % 16 == 0
    w = np.zeros((16, n // 16), dtype=np.int16)
    w[np.arange(n) % 16, np.arange(n) // 16] = v
    return np.tile(w, (8, 1))


def _match_shard(tid, src, w, tps):
    """Greedy pair matching for one (core, shard).

    Returns (descs, pairs): descs[t] is a list of (pair_id, w_lo, w_hi)
    descriptor entries for target t; pairs is the stored pair list
    [(a, b|None), ...].  Rows a target cannot pair normally (mated to a row
    outside the target) are DUPLICATED into extra pairs so nearly every
    descriptor carries two useful messages — storage is cheap, descriptors
    are not.
    """
    # dedupe (target, source) summing weights
    key = tid.astype(np.int64) * (1 << 32) + src.astype(np.int64)
    uk, inv = np.unique(key, return_inverse=True)
    wsum = np.zeros(len(uk), dtype=np.float64)
    np.add.at(wsum, inv, w)
    tid_u = (uk >> 32).astype(np.int64)
    src_u = (uk & 0xFFFFFFFF).astype(np.int64)
    order = np.argsort(tid_u, kind="stable")
    tid_u, src_u, wsum = tid_u[order], src_u[order], wsum[order]
    deg = np.bincount(tid_u, minlength=tps)
    starts = np.concatenate([[0], np.cumsum(deg)]).astype(np.int64)

    mate = {}
    half = {}      # row -> (pair_id, half) of its PRIMARY storage
    pairs = []     # stored pairs, may contain duplicated rows

    def new_pair(a, b):
        pairs.append((a, b))
        return len(pairs) - 1

    descs = [None] * tps
    pending = []   # (t, desc_idx, row, w): singles of not-yet-stored rows
    for t in np.argsort(-deg, kind="stable"):
        s0, d = starts[t], deg[t]
        sl = src_u[s0 : s0 + d].tolist()
        wl = wsum[s0 : s0 + d].tolist()
        wmap = dict(zip(sl, wl))
        used = set()
        dl = []
        for a in sl:  # free rides: pairs already mated together
            if a in used:
                continue
            b = mate.get(a)
            if b is not None and b in wmap and b not in used:
                used.add(a)
                used.add(b)
                pid, ha = half[a]
                ws = [0.0, 0.0]
                ws[ha] = wmap[a]
                ws[1 - ha] = wmap[b]
                dl.append((pid, ws[0], ws[1]))
        rem = [a for a in sl if a not in used]
        unp = [a for a in rem if a not in mate]
        for i in range(0, len(unp) - 1, 2):
            a, b = unp[i], unp[i + 1]
            mate[a] = b
            mate[b] = a
            pid = new_pair(a, b)
            half[a] = (pid, 0)
            half[b] = (pid, 1)
            dl.append((pid, wmap[a], wmap[b]))
        # leftovers: the odd unpaired row plus rows mated outside this
        # target.  Duplicate-pair them so each desc still covers 2 edges.
        left = [a for a in rem if a in mate and a not in set(unp)]
        if len(unp) % 2:
            left.append(unp[-1])
        for i in range(0, len(left) - 1, 2):
            a, b = left[i], left[i + 1]
            dl.append((new_pair(a, b), wmap[a], wmap[b]))
        if len(left) % 2:
            a = left[-1]
            if a in half:
                pid, ha = half[a]
                ws = [0.0, 0.0]
                ws[ha] = wmap[a]
                dl.append((pid, ws[0], ws[1]))
            else:
                pending.append((t, len(dl), a, wmap[a]))
                dl.append(None)
        descs[t] = dl

    # primary storage for rows never mated
    allrows = np.unique(src_u)
    loners = [a for a in allrows.tolist() if a not in half]
    for i in range(0, len(loners) - 1, 2):
        a, b = loners[i], loners[i + 1]
        pid = new_pair(a, b)
        half[a] = (pid, 0)
        half[b] = (pid, 1)
    if len(loners) % 2:
        a = loners[-1]
        half[a] = (new_pair(a, None), 0)
    for t, di, a, wa in pending:
        pid, ha = half[a]
        ws = [0.0, 0.0]
        ws[ha] = wa
        descs[t][di] = (pid, ws[0], ws[1])
    return descs, pairs


def preprocess(x, source, target, edge_weights, n_nodes):
    """Matching + shared block/group schedule + per-core tensors."""
    x = np.asarray(x, dtype=np.float32)
    source = np.asarray(source).astype(np.int64)
    target = np.asarray(target).astype(np.int64)
    edge_weights = np.asarray(edge_weights).astype(np.float64)
    nt = n_nodes // N_CORES
    assert nt * N_CORES == n_nodes
    tps = nt // NSH
    assert tps * NSH == nt
    nblk = math.ceil(tps / P)

    # ---- pass 1: matching per (core, shard); shared schedule ----
    work = []          # [core][shard] dict
    d_sched = np.zeros((NSH, nblk), dtype=np.int64)
    npairs = np.zeros(NSH, dtype=np.int64)
    for k in range(N_CORES):
        lo = k * nt
        m = (target >= lo) & (target < lo + nt)
        tl = target[m] - lo
        src_k = source[m]
        w_k = edge_weights[m]
        shards = []
        for s in range(NSH):
            ms = (tl >= s * tps) & (tl < (s + 1) * tps)
            descs, pairs = _match_shard(tl[ms] - s * tps, src_k[ms], w_k[ms], tps)
            ndesc = np.array([len(dl) for dl in descs], dtype=np.int64)
            perm = np.argsort(-ndesc, kind="stable")
            nds = ndesc[perm]
            for j in range(nblk):
                hi = min((j + 1) * P, tps)
                dj = int(nds[j * P : hi].max()) if j * P < tps else 0
                d_sched[s, j] = max(d_sched[s, j], dj)
            npairs[s] = max(npairs[s], len(pairs))
            shards.append(dict(descs=descs, pairs=pairs, perm=perm))
        work.append(shards)

    S = d_sched.sum(axis=1)
    groups = []
    for s in range(NSH):
        gs, b0, width, off = [], 0, 0, 0
        for j in range(nblk):
            dj = int(d_sched[s, j])
            assert 0 < dj <= GCAP, dj
            if width + dj > GCAP:
                gs.append((b0, j - b0, off, width))
                off += width
                b0, width = j, dj
            else:
                width += dj
        gs.append((b0, nblk - b0, off, width))
        groups.append(gs)
    col_off = np.concatenate(
        [np.zeros((NSH, 1), dtype=np.int64), np.cumsum(d_sched, axis=1)], axis=1
    )

    # ---- pass 2: per-core tensors ----
    in_maps = []
    for k in range(N_CORES):
        im = {}
        for s in range(NSH):
            sh = work[k][s]
            Ss = int(S[s])
            pairs = sh["pairs"]
            assert len(pairs) <= 32767

            xc = np.zeros((int(npairs[s]), D2), dtype=np.float32)
            for pid, (a, b) in enumerate(pairs):
                xc[pid, :D] = x[a]
                if b is not None:
                    xc[pid, D:] = x[b]
            im[f"xc{s}"] = xc

            perm = sh["perm"]
            lane = np.empty(tps, dtype=np.int64)
            blk = np.empty(tps, dtype=np.int64)
            lane[perm] = np.arange(tps) % P
            blk[perm] = np.arange(tps) // P

            gidx = np.zeros((P, Ss), dtype=np.int16)   # pad -> pair 0, w 0
            gw = np.zeros((P, 2 * Ss), dtype=np.float32)  # cast to bf16 below
            for t, dl in enumerate(sh["descs"]):
                ln, o = lane[t], col_off[s, blk[t]]
                for r, (pid, w0, w1) in enumerate(dl):
                    gidx[ln, o + r] = pid
                    gw[ln, 2 * (o + r)] = w0
                    gw[ln, 2 * (o + r) + 1] = w1
            glist = gidx.T.ravel()
            im[f"idx{s}"] = np.concatenate(
                [_wrap16(glist[o * P : (o + w) * P]) for (_, _, o, w) in groups[s]],
                axis=1,
            )
            im[f"wgt{s}"] = gw.astype(ml_dtypes.bfloat16)

            sid = np.full(nblk * P, -1, dtype=np.int16)
            sid[np.arange(tps)] = (s * tps + perm).astype(np.int16)
            assert (sid[:tps] >= 0).all() and (sid[tps:] == -1).all()
            im[f"sidx{s}"] = _wrap16(sid)
        in_maps.append(im)

    return dict(
        d_sched=d_sched, groups=groups, S=S, npairs=npairs, nt=nt, tps=tps,
        nblk=nblk, in_maps=in_maps,
    )


def build_nc(d_sched, groups, S, npairs, nt, tps, nblk, bufs=3):
    nc = bacc.Bacc("TRN2", target_bir_lowering=False, debug=False,
                   num_swdge_queues=4)
    xc_t = [nc.dram_tensor(f"xc{s}", [int(npairs[s]), D2], F32, kind="ExternalInput")
            for s in range(NSH)]
    idx_t = [nc.dram_tensor(f"idx{s}", [P, 8 * int(S[s])], I16, kind="ExternalInput")
             for s in range(NSH)]
    BF16 = mybir.dt.bfloat16
    wgt_t = [nc.dram_tensor(f"wgt{s}", [P, 2 * int(S[s])], BF16, kind="ExternalInput")
             for s in range(NSH)]
    sidx_t = [nc.dram_tensor(f"sidx{s}", [P, 8 * nblk], I16, kind="ExternalInput")
              for s in range(NSH)]
    wt_t = nc.dram_tensor("wT", [D, D], F32, kind="ExternalInput")
    eye_t = nc.dram_tensor("eye", [P, P], F32, kind="ExternalInput")
    out_t = nc.dram_tensor("out", [nt, D], F32, kind="ExternalOutput")

    with tile.TileContext(nc) as tc, ExitStack() as ctx:
        const = ctx.enter_context(tc.tile_pool(name="const", bufs=1))
        gpool = ctx.enter_context(tc.tile_pool(name="gather", bufs=5))
        mpool = ctx.enter_context(tc.tile_pool(name="msg", bufs=4))
        apool = ctx.enter_context(tc.tile_pool(name="agg", bufs=8))
        tpool = ctx.enter_context(tc.tile_pool(name="aggT", bufs=8))
        dpool = ctx.enter_context(tc.tile_pool(name="delta", bufs=4))
        psum = ctx.enter_context(tc.tile_pool(name="psum", bufs=4, space="PSUM"))

        # shard-0 gather index table first so the first gather issues ASAP;
        # everything else loads behind it
        idx_sb = [None] * NSH
        idx_sb[0] = const.tile([P, 8 * int(S[0])], I16, tag="idx0", name="idx0_sb")
        nc.sync.dma_start(out=idx_sb[0][:], in_=idx_t[0].ap())
        wgt_sb = [None] * NSH
        wgt_sb[0] = const.tile([P, 2 * int(S[0])], BF16, tag="wgt0", name="wgt0_sb")
        nc.sync.dma_start(out=wgt_sb[0][:], in_=wgt_t[0].ap())
        ident = const.tile([P, P], F32, tag="eye")
        nc.sync.dma_start(out=ident[:], in_=eye_t.ap())
        wt_sb = const.tile([D, D], F32, tag="wt")
        nc.sync.dma_start(out=wt_sb[:], in_=wt_t.ap())
        sidx_sb = [None] * NSH
        for s in range(NSH):
            if s > 0:
                idx_sb[s] = const.tile([P, 8 * int(S[s])], I16, tag=f"idx{s}",
                                       name=f"idx{s}_sb")
                nc.sync.dma_start(out=idx_sb[s][:], in_=idx_t[s].ap())
                wgt_sb[s] = const.tile([P, 2 * int(S[s])], BF16, tag=f"wgt{s}",
                                       name=f"wgt{s}_sb")
                nc.sync.dma_start(out=wgt_sb[s][:], in_=wgt_t[s].ap())
            t = const.tile([P, 8 * nblk], I16, tag=f"sidx{s}")
            nc.sync.dma_start(out=t[:], in_=sidx_t[s].ap())
            sidx_sb[s] = t

        # Prime engines on the upfront loads so steady-state instructions
        # carry at most one sync wait each.
        prime = const.tile([P, 1], F32, tag="pr1")
        nc.vector.tensor_copy(out=prime[:], in_=wgt_sb[0][:, :1])
        prime2 = const.tile([P, 1], F32, tag="pr2")
        nc.scalar.activation(prime2[:], wgt_sb[0][:, :1],
                             mybir.ActivationFunctionType.Copy)
        prime3 = const.tile([P, 1], F32, tag="pr3")
        nc.gpsimd.tensor_copy(out=prime3[:], in_=wgt_sb[0][:, :1])
        prime_ps = psum.tile([P, P], F32, tag="tp")
        nc.tensor.transpose(out=prime_ps[:], in_=ident[:], identity=ident[:])
        nc.tensor.transpose(out=prime_ps[:D, :D], in_=wt_sb[:], identity=ident[:D, :D])

        regs = {}
        def nreg(v):
            if v not in regs:
                regs[v] = nc.gpsimd.to_reg(v)
            return regs[v]

        # flat group list with software pipelining: issue gather g+1 before
        # the compute of group g so the Pool engine keeps the DMA fed.
        # Groups run in reverse block order within each shard: the shard then
        # ENDS on its single-big-block group (one reduce), which shortens the
        # serial compute at shard boundaries and the kernel tail.
        flat = [(s, grp, gi == len(groups[s]) - 1)
                for s in range(NSH)
                for gi, grp in enumerate(reversed(groups[s]))]
        # split the first groups at block boundaries: the pipeline fills with
        # short transfers AND their compute starts without waiting for a full
        # 31-column tile
        def _split_entry(s, grp, last):
            b0, nb, off, width = grp
            if nb < 2:
                return [(s, grp, last)]
            nb1 = nb // 2
            w1 = int(sum(int(d_sched[s][j]) for j in range(b0, b0 + nb1)))
            return [(s, (b0, nb1, off, w1), False),
                    (s, (b0 + nb1, nb - nb1, off + w1, width - w1), last)]
        flat = [e2 for i, e in enumerate(flat)
                for e2 in (_split_entry(*e) if i < 2 else [e])]
        deltas = {}
        gtiles = {}
        scat_lo = {}
        scat_next = {s: [nblk // 3] for s in range(NSH)}

        def emit_gather(i):
            s, (b0, nb, off, width), _ = flat[i]
            g = gpool.tile([P, width * D2], F32, tag="g")
            # first gathers are halved so the pipeline fills with a short
            # first transfer instead of one long one
            splits = [(0, width)]
            for c0, c1 in splits:
                nc.gpsimd.dma_gather(
                    g[:, c0 * D2 : c1 * D2].rearrange("p (c d) -> p c d", d=D2),
                    xc_t[s].ap(),
                    idx_sb[s][:, 8 * (off + c0) : 8 * (off + c1)],
                    (c1 - c0) * P, nreg((c1 - c0) * P), D2, elem_step=D2,
                    single_packet=False,
                    queue_num=i % 3,  # rotate SWDGE queues: desc-gen for the
                                      # next gather overlaps this transfer
                )
            gtiles[i] = g

        def emit_compute(i):
            s, (b0, nb, off, width), last = flat[i]
            g = gtiles.pop(i)
            if s not in deltas:
                deltas[s] = dpool.tile([P, nblk * D], F32, tag="d",
                                       name=f"delta{s}")
            delta = deltas[s]
            # Split the weight multiply between Pool (~2.0 ns/elem) and DVE
            # (~1.1 ns/elem) so Pool-busy ≈ DVE-busy ≈ DMA-busy.
            msg = mpool.tile([P, width * D2], F32, tag="m")
            wp = (width * 45) // 100
            if wp > 0:
                nc.gpsimd.tensor_tensor(
                    out=msg[:, : wp * D2].rearrange("p (c d) -> p c d", d=D),
                    in0=g[:, : wp * D2].rearrange("p (c d) -> p c d", d=D),
                    in1=wgt_sb[s][:, 2 * off : 2 * (off + wp)].to_broadcast(
                        [P, 2 * wp, D]),
                    op=mybir.AluOpType.mult,
                )
            nc.vector.tensor_tensor(
                out=msg[:, wp * D2 :].rearrange("p (c d) -> p c d", d=D),
                in0=g[:, wp * D2 :].rearrange("p (c d) -> p c d", d=D),
                in1=wgt_sb[s][:, 2 * (off + wp) : 2 * (off + width)].to_broadcast(
                    [P, 2 * (width - wp), D]),
                op=mybir.AluOpType.mult,
            )
            # fuse consecutive equal-width blocks into one reduce instruction
            bo = 0
            j = b0
            while j < b0 + nb:
                dj = int(d_sched[s][j])
                nbq = 1
                while (j + nbq < b0 + nb and int(d_sched[s][j + nbq]) == dj
                       and nbq < 8):
                    nbq += 1
                agg = apool.tile([P, nbq * D], F32, tag="a")
                nc.vector.tensor_reduce(
                    out=agg[:].rearrange("p (b o) -> p b o", o=D),
                    in_=msg[:, bo * D2 : (bo + nbq * dj) * D2].rearrange(
                        "p (b d o) -> p b o d", o=D, b=nbq),
                    axis=mybir.AxisListType.X,
                    op=mybir.AluOpType.add,
                )
                for b in range(nbq):
                    agg_ps = psum.tile([D, P], F32, tag="tp")
                    nc.tensor.transpose(out=agg_ps[:],
                                        in_=agg[:, b * D : (b + 1) * D],
                                        identity=ident[:])
                    agg_tr = tpool.tile([D, P], F32, tag="aT")
                    nc.scalar.activation(agg_tr[:], agg_ps[:],
                                         mybir.ActivationFunctionType.Copy)
                    d_ps = psum.tile([P, D], F32, tag="mm")
                    nc.tensor.matmul(out=d_ps[:], lhsT=agg_tr[:], rhs=wt_sb[:],
                                     start=True, stop=True)
                    nc.scalar.activation(
                        delta[:, (j + b) * D : (j + b + 1) * D], d_ps[:],
                        mybir.ActivationFunctionType.Copy)
                bo += nbq * dj
                j += nbq
            # blocks run high->low across groups; each time b0 crosses a
            # threshold, blocks [b0, prev_lo) are all done - scatter them so
            # only a small final scatter remains in the kernel tail
            if s in deltas and scat_next[s] and b0 <= scat_next[s][0]:
                thresholds = scat_next[s]
                while thresholds and b0 <= thresholds[0]:
                    thresholds.pop(0)
                hi = scat_lo.get(s, nblk)
                scat_lo[s] = b0
                nvalid = min(tps - b0 * P, (hi - b0) * P)
                nc.gpsimd.dma_scatter_add(
                    out_t.ap(),
                    delta[:, b0 * D : hi * D].rearrange("p (c d) -> p c d", d=D),
                    sidx_sb[s][:, 8 * b0 : 8 * hi],
                    (hi - b0) * P, nreg(nvalid), D, elem_step=D,
                    single_packet=False,
                    queue_num=3,  # own queue: never behind a gather's ring
                )
            if last:
                lo = scat_lo.get(s, nblk)
                if lo > 0:
                    nc.gpsimd.dma_scatter_add(
                        out_t.ap(),
                        delta[:, : lo * D].rearrange("p (c d) -> p c d", d=D),
                        sidx_sb[s][:, : 8 * lo],
                        lo * P, nreg(min(tps, lo * P)), D, elem_step=D,
                        single_packet=False,
                        queue_num=3,
                    )
                del deltas[s]

        LOOKAHEAD = 3
        for i in range(len(flat)):
            emit_gather(i)
            if i >= LOOKAHEAD:
                emit_compute(i - LOOKAHEAD)
        for i in range(len(flat) - LOOKAHEAD, len(flat)):
            emit_compute(i)
    nc.compile()
    return nc


def run_gcn(x, W, edge_weights, source, target, num_nodes, trace=False, bufs=3):
    """Full-input host entry: preprocess, build, run on 8 cores, assemble."""
    n_nodes = int(num_nodes)
    pp = preprocess(x, source, target, edge_weights, n_nodes)
    nc = build_nc(pp["d_sched"], pp["groups"], pp["S"], pp["npairs"],
                  pp["nt"], pp["tps"], pp["nblk"], bufs=bufs)
    wt_np = np.ascontiguousarray(np.asarray(W).T, dtype=np.float32)
    eye = np.eye(P, dtype=np.float32)
    in_maps = []
    for k in range(N_CORES):
        im = dict(pp["in_maps"][k])
        im["wT"] = wt_np
        im["eye"] = eye
        in_maps.append(im)
    res = run_bass_kernel_spmd(nc, in_maps, core_ids=list(range(N_CORES)),
                               trace=trace)
    out = np.concatenate([res.results[k]["out"] for k in range(N_CORES)], axis=0)
    return out, res


def kernel(**inputs) -> np.ndarray:
    """Harness entry: full unsharded inputs -> full (num_nodes, 64) output."""
    out, _ = run_gcn(
        np.asarray(inputs["x"]),
        np.asarray(inputs["W"]),
        np.asarray(inputs["edge_weights"]),
        np.asarray(inputs["source"]),
        np.asarray(inputs["target"]),
        int(inputs["num_nodes"]),
        trace=False,
    )
    return out


# revision 76
# speedup vs baseline: 1.0006x; 1.0006x over previous
"""GCN message-passing Bass kernel for TRN2 (8 cores), v3.

Math: delta = segment_sum(w_e * x[src_e]) @ W^T  (transform after aggregate).

Sharding: targets split across 8 cores (12500 each), then into 4 shards of
3125 targets per core (so gather indices fit int16).  Per shard, x rows the
shard actually reads are host-compacted and stored as 512-byte PAIRS; a
host-side greedy matching pairs up rows that some target reads together, so
one DMA descriptor often delivers two useful edge messages.  Per target the
edge list becomes a descriptor list (pair descs carry two weighted halves,
singleton descs carry one, the unused half gets weight 0).

Targets are sorted by descriptor count (descending) into 128-target blocks;
block j gets d_j descriptor columns in a [128 lanes, S] grid.  One Q7
`dma_gather` instruction fills up to 31 columns (31*128 int16 indices),
amortizing the 994 ns SWDGE fixed cost; dst[i%128, i//128] = src[idx[i]]
matches the grid when the index list is ordered column-major.

The weight multiply is split ~46/54 between Pool and DVE tensor_tensor
(cost model: Pool ~2.0 ns/elem, DVE ~1.1 ns/elem) so Pool-busy ≈ DVE-busy ≈
DMA-busy; DVE reduces each block to agg [128, 64] (consecutive equal-width
blocks fused into one reduce), PE transposes and applies W^T, the Activation
engine does PSUM->SBUF copies, and staged `dma_scatter_add` calls (int16
core-local target ids, trailing -1 padding) write rows into the pre-zeroed
output.  Groups run in reverse block order per shard and gathers are issued
3 groups ahead on rotating SWDGE queues so descriptor generation overlaps
transfers; the first gathers are halved to fill the pipeline faster.

Device limits found empirically: dma_gather/dma_scatter_add hang above 31
destination columns, and single_packet=True deadlocks when descriptors
exceed the SWDGE ring (1024), so all use single_packet=False.
"""

import math
from contextlib import ExitStack

import ml_dtypes
import numpy as np

import concourse.bass as bass
import concourse.bacc as bacc
import concourse.mybir as mybir
import concourse.tile as tile
from concourse.bass_utils import run_bass_kernel_spmd

P = 128
N_CORES = 8
NSH = 4                # target shards per core (int16 gather-index limit)
D = 64
D2 = 2 * D             # pair row: 128 floats = 512 B
GCAP = 31              # max descriptor columns per dma_gather (device limit)
F32 = mybir.dt.float32
I16 = mybir.dt.int16


def _wrap16(v):
    """Index list -> [16, n/16] wrap (element i at [i%16, i//16]), tiled to
    128 partitions (8 Q7 core replicas)."""
    v = np.asarray(v, dtype=np.int16)
    n = len(v)
    assert n % 16 == 0
    w = np.zeros((16, n // 16), dtype=np.int16)
    w[np.arange(n) % 16, np.arange(n) // 16] = v
    return np.tile(w, (8, 1))


def _match_shard(tid, src, w, tps):
    """Greedy pair matching for one (core, shard).

    Returns (descs, pairs): descs[t] is a list of (pair_id, w_lo, w_hi)
    descriptor entries for target t; pairs is the stored pair list
    [(a, b|None), ...].  Rows a target cannot pair normally (mated to a row
    outside the target) are DUPLICATED into extra pairs so nearly every
    descriptor carries two useful messages — storage is cheap, descriptors
    are not.
    """
    # dedupe (target, source) summing weights
    key = tid.astype(np.int64) * (1 << 32) + src.astype(np.int64)
    uk, inv = np.unique(key, return_inverse=True)
    wsum = np.zeros(len(uk), dtype=np.float64)
    np.add.at(wsum, inv, w)
    tid_u = (uk >> 32).astype(np.int64)
    src_u = (uk & 0xFFFFFFFF).astype(np.int64)
    order = np.argsort(tid_u, kind="stable")
    tid_u, src_u, wsum = tid_u[order], src_u[order], wsum[order]
    deg = np.bincount(tid_u, minlength=tps)
    starts = np.concatenate([[0], np.cumsum(deg)]).astype(np.int64)

    mate = {}
    half = {}      # row -> (pair_id, half) of its PRIMARY storage
    pairs = []     # stored pairs, may contain duplicated rows

    def new_pair(a, b):
        pairs.append((a, b))
        return len(pairs) - 1

    descs = [None] * tps
    pending = []   # (t, desc_idx, row, w): singles of not-yet-stored rows
    for t in np.argsort(-deg, kind="stable"):
        s0, d = starts[t], deg[t]
        sl = src_u[s0 : s0 + d].tolist()
        wl = wsum[s0 : s0 + d].tolist()
        wmap = dict(zip(sl, wl))
        used = set()
        dl = []
        for a in sl:  # free rides: pairs already mated together
            if a in used:
                continue
            b = mate.get(a)
            if b is not None and b in wmap and b not in used:
                used.add(a)
                used.add(b)
                pid, ha = half[a]
                ws = [0.0, 0.0]
                ws[ha] = wmap[a]
                ws[1 - ha] = wmap[b]
                dl.append((pid, ws[0], ws[1]))
        rem = [a for a in sl if a not in used]
        unp = [a for a in rem if a not in mate]
        for i in range(0, len(unp) - 1, 2):
            a, b = unp[i], unp[i + 1]
            mate[a] = b
            mate[b] = a
            pid = new_pair(a, b)
            half[a] = (pid, 0)
            half[b] = (pid, 1)
            dl.append((pid, wmap[a], wmap[b]))
        # leftovers: the odd unpaired row plus rows mated outside this
        # target.  Duplicate-pair them so each desc still covers 2 edges.
        left = [a for a in rem if a in mate and a not in set(unp)]
        if len(unp) % 2:
            left.append(unp[-1])
        for i in range(0, len(left) - 1, 2):
            a, b = left[i], left[i + 1]
            dl.append((new_pair(a, b), wmap[a], wmap[b]))
        if len(left) % 2:
            a = left[-1]
            if a in half:
                pid, ha = half[a]
                ws = [0.0, 0.0]
                ws[ha] = wmap[a]
                dl.append((pid, ws[0], ws[1]))
            else:
                pending.append((t, len(dl), a, wmap[a]))
                dl.append(None)
        descs[t] = dl

    # primary storage for rows never mated
    allrows = np.unique(src_u)
    loners = [a for a in allrows.tolist() if a not in half]
    for i in range(0, len(loners) - 1, 2):
        a, b = loners[i], loners[i + 1]
        pid = new_pair(a, b)
        half[a] = (pid, 0)
        half[b] = (pid, 1)
    if len(loners) % 2:
        a = loners[-1]
        half[a] = (new_pair(a, None), 0)
    for t, di, a, wa in pending:
        pid, ha = half[a]
        ws = [0.0, 0.0]
        ws[ha] = wa
        descs[t][di] = (pid, ws[0], ws[1])
    return descs, pairs


def preprocess(x, source, target, edge_weights, n_nodes):
    """Matching + shared block/group schedule + per-core tensors."""
    x = np.asarray(x, dtype=np.float32)
    source = np.asarray(source).astype(np.int64)
    target = np.asarray(target).astype(np.int64)
    edge_weights = np.asarray(edge_weights).astype(np.float64)
    nt = n_nodes // N_CORES
    assert nt * N_CORES == n_nodes
    tps = nt // NSH
    assert tps * NSH == nt
    nblk = math.ceil(tps / P)

    # ---- pass 1: matching per (core, shard); shared schedule ----
    work = []          # [core][shard] dict
    d_sched = np.zeros((NSH, nblk), dtype=np.int64)
    npairs = np.zeros(NSH, dtype=np.int64)
    for k in range(N_CORES):
        lo = k * nt
        m = (target >= lo) & (target < lo + nt)
        tl = target[m] - lo
        src_k = source[m]
        w_k = edge_weights[m]
        shards = []
        for s in range(NSH):
            ms = (tl >= s * tps) & (tl < (s + 1) * tps)
            descs, pairs = _match_shard(tl[ms] - s * tps, src_k[ms], w_k[ms], tps)
            ndesc = np.array([len(dl) for dl in descs], dtype=np.int64)
            perm = np.argsort(-ndesc, kind="stable")
            nds = ndesc[perm]
            for j in range(nblk):
                hi = min((j + 1) * P, tps)
                dj = int(nds[j * P : hi].max()) if j * P < tps else 0
                d_sched[s, j] = max(d_sched[s, j], dj)
            npairs[s] = max(npairs[s], len(pairs))
            shards.append(dict(descs=descs, pairs=pairs, perm=perm))
        work.append(shards)

    S = d_sched.sum(axis=1)
    groups = []
    for s in range(NSH):
        gs, b0, width, off = [], 0, 0, 0
        for j in range(nblk):
            dj = int(d_sched[s, j])
            assert 0 < dj <= GCAP, dj
            if width + dj > GCAP:
                gs.append((b0, j - b0, off, width))
                off += width
                b0, width = j, dj
            else:
                width += dj
        gs.append((b0, nblk - b0, off, width))
        groups.append(gs)
    col_off = np.concatenate(
        [np.zeros((NSH, 1), dtype=np.int64), np.cumsum(d_sched, axis=1)], axis=1
    )

    # ---- pass 2: per-core tensors ----
    in_maps = []
    for k in range(N_CORES):
        im = {}
        for s in range(NSH):
            sh = work[k][s]
            Ss = int(S[s])
            pairs = sh["pairs"]
            assert len(pairs) <= 32767

            xc = np.zeros((int(npairs[s]), D2), dtype=np.float32)
            for pid, (a, b) in enumerate(pairs):
                xc[pid, :D] = x[a]
                if b is not None:
                    xc[pid, D:] = x[b]
            im[f"xc{s}"] = xc

            perm = sh["perm"]
            lane = np.empty(tps, dtype=np.int64)
            blk = np.empty(tps, dtype=np.int64)
            lane[perm] = np.arange(tps) % P
            blk[perm] = np.arange(tps) // P

            gidx = np.zeros((P, Ss), dtype=np.int16)   # pad -> pair 0, w 0
            gw = np.zeros((P, 2 * Ss), dtype=np.float32)  # cast to bf16 below
            for t, dl in enumerate(sh["descs"]):
                ln, o = lane[t], col_off[s, blk[t]]
                for r, (pid, w0, w1) in enumerate(dl):
                    gidx[ln, o + r] = pid
                    gw[ln, 2 * (o + r)] = w0
                    gw[ln, 2 * (o + r) + 1] = w1
            glist = gidx.T.ravel()
            im[f"idx{s}"] = np.concatenate(
                [_wrap16(glist[o * P : (o + w) * P]) for (_, _, o, w) in groups[s]],
                axis=1,
            )
            im[f"wgt{s}"] = gw.astype(ml_dtypes.bfloat16)

            sid = np.full(nblk * P, -1, dtype=np.int16)
            sid[np.arange(tps)] = (s * tps + perm).astype(np.int16)
            assert (sid[:tps] >= 0).all() and (sid[tps:] == -1).all()
            im[f"sidx{s}"] = _wrap16(sid)
        in_maps.append(im)

    return dict(
        d_sched=d_sched, groups=groups, S=S, npairs=npairs, nt=nt, tps=tps,
        nblk=nblk, in_maps=in_maps,
    )


def build_nc(d_sched, groups, S, npairs, nt, tps, nblk, bufs=3):
    nc = bacc.Bacc("TRN2", target_bir_lowering=False, debug=False,
                   num_swdge_queues=4)
    xc_t = [nc.dram_tensor(f"xc{s}", [int(npairs[s]), D2], F32, kind="ExternalInput")
            for s in range(NSH)]
    idx_t = [nc.dram_tensor(f"idx{s}", [P, 8 * int(S[s])], I16, kind="ExternalInput")
             for s in range(NSH)]
    BF16 = mybir.dt.bfloat16
    wgt_t = [nc.dram_tensor(f"wgt{s}", [P, 2 * int(S[s])], BF16, kind="ExternalInput")
             for s in range(NSH)]
    sidx_t = [nc.dram_tensor(f"sidx{s}", [P, 8 * nblk], I16, kind="ExternalInput")
              for s in range(NSH)]
    wt_t = nc.dram_tensor("wT", [D, D], F32, kind="ExternalInput")
    eye_t = nc.dram_tensor("eye", [P, P], F32, kind="ExternalInput")
    out_t = nc.dram_tensor("out", [nt, D], F32, kind="ExternalOutput")

    with tile.TileContext(nc) as tc, ExitStack() as ctx:
        const = ctx.enter_context(tc.tile_pool(name="const", bufs=1))
        gpool = ctx.enter_context(tc.tile_pool(name="gather", bufs=5))
        mpool = ctx.enter_context(tc.tile_pool(name="msg", bufs=4))
        apool = ctx.enter_context(tc.tile_pool(name="agg", bufs=8))
        tpool = ctx.enter_context(tc.tile_pool(name="aggT", bufs=8))
        dpool = ctx.enter_context(tc.tile_pool(name="delta", bufs=4))
        psum = ctx.enter_context(tc.tile_pool(name="psum", bufs=4, space="PSUM"))

        # shard-0 gather index table first so the first gather issues ASAP;
        # everything else loads behind it
        idx_sb = [None] * NSH
        idx_sb[0] = const.tile([P, 8 * int(S[0])], I16, tag="idx0", name="idx0_sb")
        nc.sync.dma_start(out=idx_sb[0][:], in_=idx_t[0].ap())
        wgt_sb = [None] * NSH
        wgt_sb[0] = const.tile([P, 2 * int(S[0])], BF16, tag="wgt0", name="wgt0_sb")
        nc.sync.dma_start(out=wgt_sb[0][:], in_=wgt_t[0].ap())
        ident = const.tile([P, P], F32, tag="eye")
        nc.sync.dma_start(out=ident[:], in_=eye_t.ap())
        wt_sb = const.tile([D, D], F32, tag="wt")
        nc.sync.dma_start(out=wt_sb[:], in_=wt_t.ap())
        sidx_sb = [None] * NSH
        for s in range(NSH):
            if s > 0:
                idx_sb[s] = const.tile([P, 8 * int(S[s])], I16, tag=f"idx{s}",
                                       name=f"idx{s}_sb")
                nc.sync.dma_start(out=idx_sb[s][:], in_=idx_t[s].ap())
                wgt_sb[s] = const.tile([P, 2 * int(S[s])], BF16, tag=f"wgt{s}",
                                       name=f"wgt{s}_sb")
                nc.sync.dma_start(out=wgt_sb[s][:], in_=wgt_t[s].ap())
            t = const.tile([P, 8 * nblk], I16, tag=f"sidx{s}")
            nc.sync.dma_start(out=t[:], in_=sidx_t[s].ap())
            sidx_sb[s] = t

        # Prime engines on the upfront loads so steady-state instructions
        # carry at most one sync wait each.
        prime = const.tile([P, 1], F32, tag="pr1")
        nc.vector.tensor_copy(out=prime[:], in_=wgt_sb[0][:, :1])
        prime2 = const.tile([P, 1], F32, tag="pr2")
        nc.scalar.activation(prime2[:], wgt_sb[0][:, :1],
                             mybir.ActivationFunctionType.Copy)
        prime3 = const.tile([P, 1], F32, tag="pr3")
        nc.gpsimd.tensor_copy(out=prime3[:], in_=wgt_sb[0][:, :1])
        prime_ps = psum.tile([P, P], F32, tag="tp")
        nc.tensor.transpose(out=prime_ps[:], in_=ident[:], identity=ident[:])
        nc.tensor.transpose(out=prime_ps[:D, :D], in_=wt_sb[:], identity=ident[:D, :D])

        regs = {}
        def nreg(v):
            if v not in regs:
                regs[v] = nc.gpsimd.to_reg(v)
            return regs[v]

        # flat group list with software pipelining: issue gather g+1 before
        # the compute of group g so the Pool engine keeps the DMA fed.
        # Groups run in reverse block order within each shard: the shard then
        # ENDS on its single-big-block group (one reduce), which shortens the
        # serial compute at shard boundaries and the kernel tail.
        flat = [(s, grp, gi == len(groups[s]) - 1)
                for s in range(NSH)
                for gi, grp in enumerate(reversed(groups[s]))]
        # split the first groups at block boundaries: the pipeline fills with
        # short transfers AND their compute starts without waiting for a full
        # 31-column tile
        def _split_entry(s, grp, last):
            b0, nb, off, width = grp
            if nb < 2:
                return [(s, grp, last)]
            nb1 = nb // 2
            w1 = int(sum(int(d_sched[s][j]) for j in range(b0, b0 + nb1)))
            return [(s, (b0, nb1, off, w1), False),
                    (s, (b0 + nb1, nb - nb1, off + w1, width - w1), last)]
        flat = [e2 for i, e in enumerate(flat)
                for e2 in (_split_entry(*e) if i < 2 else [e])]
        deltas = {}
        gtiles = {}
        scat_lo = {}
        scat_next = {s: [nblk // 3] for s in range(NSH)}

        def emit_gather(i):
            s, (b0, nb, off, width), _ = flat[i]
            g = gpool.tile([P, width * D2], F32, tag="g")
            # first gathers are halved so the pipeline fills with a short
            # first transfer instead of one long one
            splits = [(0, width)]
            for c0, c1 in splits:
                nc.gpsimd.dma_gather(
                    g[:, c0 * D2 : c1 * D2].rearrange("p (c d) -> p c d", d=D2),
                    xc_t[s].ap(),
                    idx_sb[s][:, 8 * (off + c0) : 8 * (off + c1)],
                    (c1 - c0) * P, nreg((c1 - c0) * P), D2, elem_step=D2,
                    single_packet=False,
                    queue_num=i % 3,  # rotate SWDGE queues: desc-gen for the
                                      # next gather overlaps this transfer
                )
            gtiles[i] = g

        def emit_compute(i):
            s, (b0, nb, off, width), last = flat[i]
            g = gtiles.pop(i)
            if s not in deltas:
                deltas[s] = dpool.tile([P, nblk * D], F32, tag="d",
                                       name=f"delta{s}")
            delta = deltas[s]
            # Split the weight multiply between Pool (~2.0 ns/elem) and DVE
            # (~1.1 ns/elem) so Pool-busy ≈ DVE-busy ≈ DMA-busy.
            msg = mpool.tile([P, width * D2], F32, tag="m")
            wp = (width * 45) // 100
            if wp > 0:
                nc.gpsimd.tensor_tensor(
                    out=msg[:, : wp * D2].rearrange("p (c d) -> p c d", d=D),
                    in0=g[:, : wp * D2].rearrange("p (c d) -> p c d", d=D),
                    in1=wgt_sb[s][:, 2 * off : 2 * (off + wp)].to_broadcast(
                        [P, 2 * wp, D]),
                    op=mybir.AluOpType.mult,
                )
            nc.vector.tensor_tensor(
                out=msg[:, wp * D2 :].rearrange("p (c d) -> p c d", d=D),
                in0=g[:, wp * D2 :].rearrange("p (c d) -> p c d", d=D),
                in1=wgt_sb[s][:, 2 * (off + wp) : 2 * (off + width)].to_broadcast(
                    [P, 2 * (width - wp), D]),
                op=mybir.AluOpType.mult,
            )
            # fuse consecutive equal-width blocks into one reduce instruction
            bo = 0
            j = b0
            while j < b0 + nb:
                dj = int(d_sched[s][j])
                nbq = 1
                while (j + nbq < b0 + nb and int(d_sched[s][j + nbq]) == dj
                       and nbq < 8):
                    nbq += 1
                agg = apool.tile([P, nbq * D], F32, tag="a")
                nc.vector.tensor_reduce(
                    out=agg[:].rearrange("p (b o) -> p b o", o=D),
                    in_=msg[:, bo * D2 : (bo + nbq * dj) * D2].rearrange(
                        "p (b d o) -> p b o d", o=D, b=nbq),
                    axis=mybir.AxisListType.X,
                    op=mybir.AluOpType.add,
                )
                for b in range(nbq):
                    agg_ps = psum.tile([D, P], F32, tag="tp")
                    nc.tensor.transpose(out=agg_ps[:],
                                        in_=agg[:, b * D : (b + 1) * D],
                                        identity=ident[:])
                    agg_tr = tpool.tile([D, P], F32, tag="aT")
                    nc.scalar.activation(agg_tr[:], agg_ps[:],
                                         mybir.ActivationFunctionType.Copy)
                    d_ps = psum.tile([P, D], F32, tag="mm")
                    nc.tensor.matmul(out=d_ps[:], lhsT=agg_tr[:], rhs=wt_sb[:],
                                     start=True, stop=True)
                    nc.scalar.activation(
                        delta[:, (j + b) * D : (j + b + 1) * D], d_ps[:],
                        mybir.ActivationFunctionType.Copy)
                bo += nbq * dj
                j += nbq
            # blocks run high->low across groups; each time b0 crosses a
            # threshold, blocks [b0, prev_lo) are all done - scatter them so
            # only a small final scatter remains in the kernel tail
            if s in deltas and scat_next[s] and b0 <= scat_next[s][0]:
                thresholds = scat_next[s]
                while thresholds and b0 <= thresholds[0]:
                    thresholds.pop(0)
                hi = scat_lo.get(s, nblk)
                scat_lo[s] = b0
                nvalid = min(tps - b0 * P, (hi - b0) * P)
                nc.gpsimd.dma_scatter_add(
                    out_t.ap(),
                    delta[:, b0 * D : hi * D].rearrange("p (c d) -> p c d", d=D),
                    sidx_sb[s][:, 8 * b0 : 8 * hi],
                    (hi - b0) * P, nreg(nvalid), D, elem_step=D,
                    single_packet=False,
                    queue_num=3,  # own queue: never behind a gather's ring
                )
            if last:
                lo = scat_lo.get(s, nblk)
                if lo > 0:
                    nc.gpsimd.dma_scatter_add(
                        out_t.ap(),
                        delta[:, : lo * D].rearrange("p (c d) -> p c d", d=D),
                        sidx_sb[s][:, : 8 * lo],
                        lo * P, nreg(min(tps, lo * P)), D, elem_step=D,
                        single_packet=False,
                        queue_num=3,
                    )
                del deltas[s]

        LOOKAHEAD = 5
        for i in range(len(flat)):
            emit_gather(i)
            if i >= LOOKAHEAD:
                emit_compute(i - LOOKAHEAD)
        for i in range(len(flat) - LOOKAHEAD, len(flat)):
            emit_compute(i)
    nc.compile()
    return nc


def run_gcn(x, W, edge_weights, source, target, num_nodes, trace=False, bufs=3):
    """Full-input host entry: preprocess, build, run on 8 cores, assemble."""
    n_nodes = int(num_nodes)
    pp = preprocess(x, source, target, edge_weights, n_nodes)
    nc = build_nc(pp["d_sched"], pp["groups"], pp["S"], pp["npairs"],
                  pp["nt"], pp["tps"], pp["nblk"], bufs=bufs)
    wt_np = np.ascontiguousarray(np.asarray(W).T, dtype=np.float32)
    eye = np.eye(P, dtype=np.float32)
    in_maps = []
    for k in range(N_CORES):
        im = dict(pp["in_maps"][k])
        im["wT"] = wt_np
        im["eye"] = eye
        in_maps.append(im)
    res = run_bass_kernel_spmd(nc, in_maps, core_ids=list(range(N_CORES)),
                               trace=trace)
    out = np.concatenate([res.results[k]["out"] for k in range(N_CORES)], axis=0)
    return out, res


def kernel(**inputs) -> np.ndarray:
    """Harness entry: full unsharded inputs -> full (num_nodes, 64) output."""
    out, _ = run_gcn(
        np.asarray(inputs["x"]),
        np.asarray(inputs["W"]),
        np.asarray(inputs["edge_weights"]),
        np.asarray(inputs["source"]),
        np.asarray(inputs["target"]),
        int(inputs["num_nodes"]),
        trace=False,
    )
    return out


# revision 82
# speedup vs baseline: 1.0038x; 1.0033x over previous
"""GCN message-passing Bass kernel for TRN2 (8 cores), v3.

Math: delta = segment_sum(w_e * x[src_e]) @ W^T  (transform after aggregate).

Sharding: targets split across 8 cores (12500 each), then into 4 shards of
3125 targets per core (so gather indices fit int16).  Per shard, x rows the
shard actually reads are host-compacted and stored as 512-byte PAIRS; a
host-side greedy matching pairs up rows that some target reads together, so
one DMA descriptor often delivers two useful edge messages.  Per target the
edge list becomes a descriptor list (pair descs carry two weighted halves,
singleton descs carry one, the unused half gets weight 0).

Targets are sorted by descriptor count (descending) into 128-target blocks;
block j gets d_j descriptor columns in a [128 lanes, S] grid.  One Q7
`dma_gather` instruction fills up to 31 columns (31*128 int16 indices),
amortizing the 994 ns SWDGE fixed cost; dst[i%128, i//128] = src[idx[i]]
matches the grid when the index list is ordered column-major.

The weight multiply is split ~46/54 between Pool and DVE tensor_tensor
(cost model: Pool ~2.0 ns/elem, DVE ~1.1 ns/elem) so Pool-busy ≈ DVE-busy ≈
DMA-busy; DVE reduces each block to agg [128, 64] (consecutive equal-width
blocks fused into one reduce), PE transposes and applies W^T, the Activation
engine does PSUM->SBUF copies, and staged `dma_scatter_add` calls (int16
core-local target ids, trailing -1 padding) write rows into the pre-zeroed
output.  Groups run in reverse block order per shard and gathers are issued
3 groups ahead on rotating SWDGE queues so descriptor generation overlaps
transfers; the first gathers are halved to fill the pipeline faster.

Device limits found empirically: dma_gather/dma_scatter_add hang above 31
destination columns, and single_packet=True deadlocks when descriptors
exceed the SWDGE ring (1024), so all use single_packet=False.
"""

import math
from contextlib import ExitStack

import ml_dtypes
import numpy as np

import concourse.bass as bass
import concourse.bacc as bacc
import concourse.mybir as mybir
import concourse.tile as tile
from concourse.bass_utils import run_bass_kernel_spmd

P = 128
N_CORES = 8
NSH = 4                # target shards per core (int16 gather-index limit)
D = 64
D2 = 2 * D             # pair row: 128 floats = 512 B
GCAP = 31              # max descriptor columns per dma_gather (device limit)
F32 = mybir.dt.float32
I16 = mybir.dt.int16


def _wrap16(v):
    """Index list -> [16, n/16] wrap (element i at [i%16, i//16]), tiled to
    128 partitions (8 Q7 core replicas)."""
    v = np.asarray(v, dtype=np.int16)
    n = len(v)
    assert n % 16 == 0
    w = np.zeros((16, n // 16), dtype=np.int16)
    w[np.arange(n) % 16, np.arange(n) // 16] = v
    return np.tile(w, (8, 1))


def _match_shard(tid, src, w, tps):
    """Greedy pair matching for one (core, shard).

    Returns (descs, pairs): descs[t] is a list of (pair_id, w_lo, w_hi)
    descriptor entries for target t; pairs is the stored pair list
    [(a, b|None), ...].  Rows a target cannot pair normally (mated to a row
    outside the target) are DUPLICATED into extra pairs so nearly every
    descriptor carries two useful messages — storage is cheap, descriptors
    are not.
    """
    # dedupe (target, source) summing weights
    key = tid.astype(np.int64) * (1 << 32) + src.astype(np.int64)
    uk, inv = np.unique(key, return_inverse=True)
    wsum = np.zeros(len(uk), dtype=np.float64)
    np.add.at(wsum, inv, w)
    tid_u = (uk >> 32).astype(np.int64)
    src_u = (uk & 0xFFFFFFFF).astype(np.int64)
    order = np.argsort(tid_u, kind="stable")
    tid_u, src_u, wsum = tid_u[order], src_u[order], wsum[order]
    deg = np.bincount(tid_u, minlength=tps)
    starts = np.concatenate([[0], np.cumsum(deg)]).astype(np.int64)

    mate = {}
    half = {}      # row -> (pair_id, half) of its PRIMARY storage
    pairs = []     # stored pairs, may contain duplicated rows

    def new_pair(a, b):
        pairs.append((a, b))
        return len(pairs) - 1

    descs = [None] * tps
    pending = []   # (t, desc_idx, row, w): singles of not-yet-stored rows
    for t in np.argsort(-deg, kind="stable"):
        s0, d = starts[t], deg[t]
        sl = src_u[s0 : s0 + d].tolist()
        wl = wsum[s0 : s0 + d].tolist()
        wmap = dict(zip(sl, wl))
        used = set()
        dl = []
        for a in sl:  # free rides: pairs already mated together
            if a in used:
                continue
            b = mate.get(a)
            if b is not None and b in wmap and b not in used:
                used.add(a)
                used.add(b)
                pid, ha = half[a]
                ws = [0.0, 0.0]
                ws[ha] = wmap[a]
                ws[1 - ha] = wmap[b]
                dl.append((pid, ws[0], ws[1]))
        rem = [a for a in sl if a not in used]
        unp = [a for a in rem if a not in mate]
        for i in range(0, len(unp) - 1, 2):
            a, b = unp[i], unp[i + 1]
            mate[a] = b
            mate[b] = a
            pid = new_pair(a, b)
            half[a] = (pid, 0)
            half[b] = (pid, 1)
            dl.append((pid, wmap[a], wmap[b]))
        # leftovers: the odd unpaired row plus rows mated outside this
        # target.  Duplicate-pair them so each desc still covers 2 edges.
        left = [a for a in rem if a in mate and a not in set(unp)]
        if len(unp) % 2:
            left.append(unp[-1])
        for i in range(0, len(left) - 1, 2):
            a, b = left[i], left[i + 1]
            dl.append((new_pair(a, b), wmap[a], wmap[b]))
        if len(left) % 2:
            a = left[-1]
            if a in half:
                pid, ha = half[a]
                ws = [0.0, 0.0]
                ws[ha] = wmap[a]
                dl.append((pid, ws[0], ws[1]))
            else:
                pending.append((t, len(dl), a, wmap[a]))
                dl.append(None)
        descs[t] = dl

    # primary storage for rows never mated
    allrows = np.unique(src_u)
    loners = [a for a in allrows.tolist() if a not in half]
    for i in range(0, len(loners) - 1, 2):
        a, b = loners[i], loners[i + 1]
        pid = new_pair(a, b)
        half[a] = (pid, 0)
        half[b] = (pid, 1)
    if len(loners) % 2:
        a = loners[-1]
        half[a] = (new_pair(a, None), 0)
    for t, di, a, wa in pending:
        pid, ha = half[a]
        ws = [0.0, 0.0]
        ws[ha] = wa
        descs[t][di] = (pid, ws[0], ws[1])
    return descs, pairs


def preprocess(x, source, target, edge_weights, n_nodes):
    """Matching + shared block/group schedule + per-core tensors."""
    x = np.asarray(x, dtype=np.float32)
    source = np.asarray(source).astype(np.int64)
    target = np.asarray(target).astype(np.int64)
    edge_weights = np.asarray(edge_weights).astype(np.float64)
    nt = n_nodes // N_CORES
    assert nt * N_CORES == n_nodes
    tps = nt // NSH
    assert tps * NSH == nt
    nblk = math.ceil(tps / P)

    # ---- pass 1: matching per (core, shard); shared schedule ----
    work = []          # [core][shard] dict
    d_sched = np.zeros((NSH, nblk), dtype=np.int64)
    npairs = np.zeros(NSH, dtype=np.int64)
    for k in range(N_CORES):
        lo = k * nt
        m = (target >= lo) & (target < lo + nt)
        tl = target[m] - lo
        src_k = source[m]
        w_k = edge_weights[m]
        shards = []
        for s in range(NSH):
            ms = (tl >= s * tps) & (tl < (s + 1) * tps)
            descs, pairs = _match_shard(tl[ms] - s * tps, src_k[ms], w_k[ms], tps)
            ndesc = np.array([len(dl) for dl in descs], dtype=np.int64)
            perm = np.argsort(-ndesc, kind="stable")
            nds = ndesc[perm]
            for j in range(nblk):
                hi = min((j + 1) * P, tps)
                dj = int(nds[j * P : hi].max()) if j * P < tps else 0
                d_sched[s, j] = max(d_sched[s, j], dj)
            npairs[s] = max(npairs[s], len(pairs))
            shards.append(dict(descs=descs, pairs=pairs, perm=perm))
        work.append(shards)

    S = d_sched.sum(axis=1)
    groups = []
    for s in range(NSH):
        gs, b0, width, off = [], 0, 0, 0
        for j in range(nblk):
            dj = int(d_sched[s, j])
            assert 0 < dj <= GCAP, dj
            if width + dj > GCAP:
                gs.append((b0, j - b0, off, width))
                off += width
                b0, width = j, dj
            else:
                width += dj
        gs.append((b0, nblk - b0, off, width))
        groups.append(gs)
    col_off = np.concatenate(
        [np.zeros((NSH, 1), dtype=np.int64), np.cumsum(d_sched, axis=1)], axis=1
    )

    # ---- pass 2: per-core tensors ----
    in_maps = []
    for k in range(N_CORES):
        im = {}
        for s in range(NSH):
            sh = work[k][s]
            Ss = int(S[s])
            pairs = sh["pairs"]
            assert len(pairs) <= 32767

            xc = np.zeros((int(npairs[s]), D2), dtype=np.float32)
            for pid, (a, b) in enumerate(pairs):
                xc[pid, :D] = x[a]
                if b is not None:
                    xc[pid, D:] = x[b]
            im[f"xc{s}"] = xc

            perm = sh["perm"]
            lane = np.empty(tps, dtype=np.int64)
            blk = np.empty(tps, dtype=np.int64)
            lane[perm] = np.arange(tps) % P
            blk[perm] = np.arange(tps) // P

            gidx = np.zeros((P, Ss), dtype=np.int16)   # pad -> pair 0, w 0
            gw = np.zeros((P, 2 * Ss), dtype=np.float32)  # cast to bf16 below
            for t, dl in enumerate(sh["descs"]):
                ln, o = lane[t], col_off[s, blk[t]]
                for r, (pid, w0, w1) in enumerate(dl):
                    gidx[ln, o + r] = pid
                    gw[ln, 2 * (o + r)] = w0
                    gw[ln, 2 * (o + r) + 1] = w1
            glist = gidx.T.ravel()
            im[f"idx{s}"] = np.concatenate(
                [_wrap16(glist[o * P : (o + w) * P]) for (_, _, o, w) in groups[s]],
                axis=1,
            )
            im[f"wgt{s}"] = gw.astype(ml_dtypes.bfloat16)

            sid = np.full(nblk * P, -1, dtype=np.int16)
            sid[np.arange(tps)] = (s * tps + perm).astype(np.int16)
            assert (sid[:tps] >= 0).all() and (sid[tps:] == -1).all()
            im[f"sidx{s}"] = _wrap16(sid)
        in_maps.append(im)

    return dict(
        d_sched=d_sched, groups=groups, S=S, npairs=npairs, nt=nt, tps=tps,
        nblk=nblk, in_maps=in_maps,
    )


def build_nc(d_sched, groups, S, npairs, nt, tps, nblk, bufs=3):
    nc = bacc.Bacc("TRN2", target_bir_lowering=False, debug=False,
                   num_swdge_queues=4)
    xc_t = [nc.dram_tensor(f"xc{s}", [int(npairs[s]), D2], F32, kind="ExternalInput")
            for s in range(NSH)]
    idx_t = [nc.dram_tensor(f"idx{s}", [P, 8 * int(S[s])], I16, kind="ExternalInput")
             for s in range(NSH)]
    BF16 = mybir.dt.bfloat16
    wgt_t = [nc.dram_tensor(f"wgt{s}", [P, 2 * int(S[s])], BF16, kind="ExternalInput")
             for s in range(NSH)]
    sidx_t = [nc.dram_tensor(f"sidx{s}", [P, 8 * nblk], I16, kind="ExternalInput")
              for s in range(NSH)]
    wt_t = nc.dram_tensor("wT", [D, D], F32, kind="ExternalInput")
    eye_t = nc.dram_tensor("eye", [P, P], F32, kind="ExternalInput")
    out_t = nc.dram_tensor("out", [nt, D], F32, kind="ExternalOutput")

    with tile.TileContext(nc) as tc, ExitStack() as ctx:
        const = ctx.enter_context(tc.tile_pool(name="const", bufs=1))
        gpool = ctx.enter_context(tc.tile_pool(name="gather", bufs=5))
        mpool = ctx.enter_context(tc.tile_pool(name="msg", bufs=4))
        apool = ctx.enter_context(tc.tile_pool(name="agg", bufs=8))
        tpool = ctx.enter_context(tc.tile_pool(name="aggT", bufs=8))
        dpool = ctx.enter_context(tc.tile_pool(name="delta", bufs=4))
        psum = ctx.enter_context(tc.tile_pool(name="psum", bufs=4, space="PSUM"))

        # shard-0 gather index table first so the first gather issues ASAP;
        # everything else loads behind it
        idx_sb = [None] * NSH
        idx_sb[0] = const.tile([P, 8 * int(S[0])], I16, tag="idx0", name="idx0_sb")
        nc.sync.dma_start(out=idx_sb[0][:], in_=idx_t[0].ap())
        wgt_sb = [None] * NSH
        wgt_sb[0] = const.tile([P, 2 * int(S[0])], BF16, tag="wgt0", name="wgt0_sb")
        nc.sync.dma_start(out=wgt_sb[0][:], in_=wgt_t[0].ap())
        ident = const.tile([P, P], F32, tag="eye")
        nc.sync.dma_start(out=ident[:], in_=eye_t.ap())
        wt_sb = const.tile([D, D], F32, tag="wt")
        nc.sync.dma_start(out=wt_sb[:], in_=wt_t.ap())
        sidx_sb = [None] * NSH
        for s in range(NSH):
            if s > 0:
                idx_sb[s] = const.tile([P, 8 * int(S[s])], I16, tag=f"idx{s}",
                                       name=f"idx{s}_sb")
                nc.sync.dma_start(out=idx_sb[s][:], in_=idx_t[s].ap())
                wgt_sb[s] = const.tile([P, 2 * int(S[s])], BF16, tag=f"wgt{s}",
                                       name=f"wgt{s}_sb")
                nc.sync.dma_start(out=wgt_sb[s][:], in_=wgt_t[s].ap())
            t = const.tile([P, 8 * nblk], I16, tag=f"sidx{s}")
            nc.sync.dma_start(out=t[:], in_=sidx_t[s].ap())
            sidx_sb[s] = t


        regs = {}
        def nreg(v):
            if v not in regs:
                regs[v] = nc.gpsimd.to_reg(v)
            return regs[v]

        # flat group list with software pipelining: issue gather g+1 before
        # the compute of group g so the Pool engine keeps the DMA fed.
        # Groups run in reverse block order within each shard: the shard then
        # ENDS on its single-big-block group (one reduce), which shortens the
        # serial compute at shard boundaries and the kernel tail.
        flat = [(s, grp, gi == len(groups[s]) - 1)
                for s in range(NSH)
                for gi, grp in enumerate(reversed(groups[s]))]
        # split the first groups at block boundaries: the pipeline fills with
        # short transfers AND their compute starts without waiting for a full
        # 31-column tile
        def _split_entry(s, grp, last):
            b0, nb, off, width = grp
            if nb < 2:
                return [(s, grp, last)]
            nb1 = nb // 2
            w1 = int(sum(int(d_sched[s][j]) for j in range(b0, b0 + nb1)))
            return [(s, (b0, nb1, off, w1), False),
                    (s, (b0 + nb1, nb - nb1, off + w1, width - w1), last)]
        flat = [e2 for i, e in enumerate(flat)
                for e2 in (_split_entry(*e) if i < 2 else [e])]
        deltas = {}
        gtiles = {}
        scat_lo = {}
        scat_next = {s: [nblk // 3] for s in range(NSH)}

        def emit_gather(i):
            s, (b0, nb, off, width), _ = flat[i]
            g = gpool.tile([P, width * D2], F32, tag="g")
            # first gathers are halved so the pipeline fills with a short
            # first transfer instead of one long one
            splits = [(0, width)]
            for c0, c1 in splits:
                nc.gpsimd.dma_gather(
                    g[:, c0 * D2 : c1 * D2].rearrange("p (c d) -> p c d", d=D2),
                    xc_t[s].ap(),
                    idx_sb[s][:, 8 * (off + c0) : 8 * (off + c1)],
                    (c1 - c0) * P, nreg((c1 - c0) * P), D2, elem_step=D2,
                    single_packet=False,
                    queue_num=i % 3,  # rotate SWDGE queues: desc-gen for the
                                      # next gather overlaps this transfer
                )
            gtiles[i] = g

        def emit_compute(i):
            s, (b0, nb, off, width), last = flat[i]
            g = gtiles.pop(i)
            if s not in deltas:
                deltas[s] = dpool.tile([P, nblk * D], F32, tag="d",
                                       name=f"delta{s}")
            delta = deltas[s]
            # Split the weight multiply between Pool (~2.0 ns/elem) and DVE
            # (~1.1 ns/elem) so Pool-busy ≈ DVE-busy ≈ DMA-busy.
            msg = mpool.tile([P, width * D2], F32, tag="m")
            wp = (width * 46) // 100
            if wp > 0:
                nc.gpsimd.tensor_tensor(
                    out=msg[:, : wp * D2].rearrange("p (c d) -> p c d", d=D),
                    in0=g[:, : wp * D2].rearrange("p (c d) -> p c d", d=D),
                    in1=wgt_sb[s][:, 2 * off : 2 * (off + wp)].to_broadcast(
                        [P, 2 * wp, D]),
                    op=mybir.AluOpType.mult,
                )
            nc.vector.tensor_tensor(
                out=msg[:, wp * D2 :].rearrange("p (c d) -> p c d", d=D),
                in0=g[:, wp * D2 :].rearrange("p (c d) -> p c d", d=D),
                in1=wgt_sb[s][:, 2 * (off + wp) : 2 * (off + width)].to_broadcast(
                    [P, 2 * (width - wp), D]),
                op=mybir.AluOpType.mult,
            )
            # fuse consecutive equal-width blocks into one reduce instruction
            bo = 0
            j = b0
            while j < b0 + nb:
                dj = int(d_sched[s][j])
                nbq = 1
                while (j + nbq < b0 + nb and int(d_sched[s][j + nbq]) == dj
                       and nbq < 8):
                    nbq += 1
                agg = apool.tile([P, nbq * D], F32, tag="a")
                nc.vector.tensor_reduce(
                    out=agg[:].rearrange("p (b o) -> p b o", o=D),
                    in_=msg[:, bo * D2 : (bo + nbq * dj) * D2].rearrange(
                        "p (b d o) -> p b o d", o=D, b=nbq),
                    axis=mybir.AxisListType.X,
                    op=mybir.AluOpType.add,
                )
                for b in range(nbq):
                    agg_ps = psum.tile([D, P], F32, tag="tp")
                    nc.tensor.transpose(out=agg_ps[:],
                                        in_=agg[:, b * D : (b + 1) * D],
                                        identity=ident[:])
                    agg_tr = tpool.tile([D, P], F32, tag="aT")
                    nc.scalar.activation(agg_tr[:], agg_ps[:],
                                         mybir.ActivationFunctionType.Copy)
                    d_ps = psum.tile([P, D], F32, tag="mm")
                    nc.tensor.matmul(out=d_ps[:], lhsT=agg_tr[:], rhs=wt_sb[:],
                                     start=True, stop=True)
                    nc.scalar.activation(
                        delta[:, (j + b) * D : (j + b + 1) * D], d_ps[:],
                        mybir.ActivationFunctionType.Copy)
                bo += nbq * dj
                j += nbq
            # blocks run high->low across groups; each time b0 crosses a
            # threshold, blocks [b0, prev_lo) are all done - scatter them so
            # only a small final scatter remains in the kernel tail
            if s in deltas and scat_next[s] and b0 <= scat_next[s][0]:
                thresholds = scat_next[s]
                while thresholds and b0 <= thresholds[0]:
                    thresholds.pop(0)
                hi = scat_lo.get(s, nblk)
                scat_lo[s] = b0
                nvalid = min(tps - b0 * P, (hi - b0) * P)
                nc.gpsimd.dma_scatter_add(
                    out_t.ap(),
                    delta[:, b0 * D : hi * D].rearrange("p (c d) -> p c d", d=D),
                    sidx_sb[s][:, 8 * b0 : 8 * hi],
                    (hi - b0) * P, nreg(nvalid), D, elem_step=D,
                    single_packet=False,
                    queue_num=3,  # own queue: never behind a gather's ring
                )
            if last:
                lo = scat_lo.get(s, nblk)
                if lo > 0:
                    nc.gpsimd.dma_scatter_add(
                        out_t.ap(),
                        delta[:, : lo * D].rearrange("p (c d) -> p c d", d=D),
                        sidx_sb[s][:, : 8 * lo],
                        lo * P, nreg(min(tps, lo * P)), D, elem_step=D,
                        single_packet=False,
                        queue_num=3,
                    )
                del deltas[s]

        LOOKAHEAD = 5
        for i in range(len(flat)):
            emit_gather(i)
            if i >= LOOKAHEAD:
                emit_compute(i - LOOKAHEAD)
        for i in range(len(flat) - LOOKAHEAD, len(flat)):
            emit_compute(i)
    nc.compile()
    return nc


def run_gcn(x, W, edge_weights, source, target, num_nodes, trace=False, bufs=3):
    """Full-input host entry: preprocess, build, run on 8 cores, assemble."""
    n_nodes = int(num_nodes)
    pp = preprocess(x, source, target, edge_weights, n_nodes)
    nc = build_nc(pp["d_sched"], pp["groups"], pp["S"], pp["npairs"],
                  pp["nt"], pp["tps"], pp["nblk"], bufs=bufs)
    wt_np = np.ascontiguousarray(np.asarray(W).T, dtype=np.float32)
    eye = np.eye(P, dtype=np.float32)
    in_maps = []
    for k in range(N_CORES):
        im = dict(pp["in_maps"][k])
        im["wT"] = wt_np
        im["eye"] = eye
        in_maps.append(im)
    res = run_bass_kernel_spmd(nc, in_maps, core_ids=list(range(N_CORES)),
                               trace=trace)
    out = np.concatenate([res.results[k]["out"] for k in range(N_CORES)], axis=0)
    return out, res


def kernel(**inputs) -> np.ndarray:
    """Harness entry: full unsharded inputs -> full (num_nodes, 64) output."""
    out, _ = run_gcn(
        np.asarray(inputs["x"]),
        np.asarray(inputs["W"]),
        np.asarray(inputs["edge_weights"]),
        np.asarray(inputs["source"]),
        np.asarray(inputs["target"]),
        int(inputs["num_nodes"]),
        trace=False,
    )
    return out
